# revision 16
# baseline (speedup 1.0000x reference)
"""Trainium2 Bass kernel for nn_Decoder_81063212745440.

Pointer-network-style decoder: 4 sequential decode steps over a 4096-token
document. Each step: LSTM cell -> per-side (start/end) expert mixture with
maxout + HMN head -> per-position logits -> argmax feeds the next step.

Distribution: document dim m=4096 sharded across 8 cores (512 rows each).
LSTM/controller state is replicated on every core. Per step, each core
computes its 512 local logits; an AllGather of per-core (max, argmax) pairs
lets every core compute the global argmax; the selected U rows are fetched
by dynamic-offset DMA from a DRAM copy of U.T.

Key algebraic restructuring: the expert input is concat(U, broadcast(r)),
so  z = U_loc @ We[:, :2d].T  +  We[:, 2d:] @ r  + be.  The first (heavy)
term is step-invariant and is precomputed once; per-step work is only the
rank-1 v-term, bias + maxout, and the small HMN GEMMs.

Big GEMMs run as float32r (TF32-like) at full PE rate; fp32 storage
everywhere keeps logit error ~1e-3, far below the argmax margins.
"""

import numpy as np

import concourse.bacc as bacc
import concourse.bass as bass
import concourse.mybir as mybir
import concourse.tile as tile
from concourse import bass_utils

D = 128          # hidden dim d
P = 8            # maxout pool width
K = 2            # experts
STEPS = 4
M = 4096         # document length
NCORES = 8
MLOC = M // NCORES   # 512 rows per core
F32 = mybir.dt.float32
F32R = mybir.dt.float32r
I32 = mybir.dt.int32
X = mybir.AxisListType.X
ALU = mybir.AluOpType
ACTF = mybir.ActivationFunctionType
ET = mybir.EngineType
BIG = 1.0e9
SIDES = ("s", "e")

_CACHE = {}


def _build(mode="full"):
    """Build the SPMD Bass program (identical on all cores; data differs)."""
    nc = bacc.Bacc("TRN2", target_bir_lowering=False, debug=False,
                   num_devices=NCORES)

    # ---- I/O declarations ----------------------------------------------
    inp = {}

    def din(name, shape, dt=F32):
        inp[name] = nc.dram_tensor(name, list(shape), dt, kind="ExternalInput")
        return inp[name]

    # f32r tensors feed the PE; rounding happens on PE operand read.
    din("uTd", (128, 2 * M), F32R)     # U.T packed: [p, j*M+m] = U[m, j*128+p]
    din("ulocT", (128, 2 * MLOC), F32R)  # per-core U slice, same layout
    din("wihT", (128, 4 * 512), F32R)  # Wih.T k-tiles
    din("whhT", (128, 512), F32R)      # Whh.T
    din("bihhR", (1, 512), F32R)       # bih + bhh row
    din("iota2d", (128, 4))            # global m idx: [p,mt]=c*512+mt*128+p
    din("onesR", (1, 512), F32R)       # ones row (f32r matmul helper)
    din("onesF", (1, 512))             # ones row (f32 matmul helper)
    din("ident", (128, 128))           # identity (PE partition transpose)
    din("zeroC", (128, 1), F32R)       # f32r zero column (initial h)
    for s in SIDES:
        din(f"weT_{s}", (128, K * 2 * P * 128), F32R)
        din(f"weR_{s}", (128, K * P * 128))       # v matvec lhsT (f32)
        din(f"beC_{s}", (128, K * P))             # be cols per (expert, ptile)
        din(f"w2T_{s}", (128, P * 128), F32R)     # W2.T lhsT tiles
        din(f"b2C_{s}", (128, P))                 # b2 cols per ptile
        din(f"w3T_{s}", (128, 2 * P), F32R)       # W3.T k-tiles
        din(f"b3F_{s}", (128, 4 * P))             # b3 bcast: [p, mt*8+pp]=b3[pp]
        din(f"wrT_{s}", (128, 5 * 128), F32R)     # Wr.T rhs k-tiles
        din(f"brR_{s}", (1, 128), F32R)
        din(f"wgT_{s}", (128, 5 * K), F32R)       # Wg.T rhs k-tiles
        din(f"bgR_{s}", (1, K), F32R)

    out_d = {s: nc.dram_tensor(f"out_{s}", [STEPS, MLOC], F32,
                               kind="ExternalOutput") for s in SIDES}

    rg = [list(range(NCORES))]
    uTd = inp["uTd"]

    with (
        tile.TileContext(nc) as tc,
        tc.tile_pool(name="consts", bufs=1) as constp,
    ):
        # ---- persistent SBUF constants ---------------------------------
        sb = {}
        for name in ("wihT", "whhT", "bihhR", "iota2d", "onesR", "onesF",
                     "ident"):
            t = constp.tile(list(inp[name].shape), inp[name].dtype,
                            tag=f"sb_{name}", name=f"sb_{name}")
            nc.sync.dma_start(t[:], inp[name][:])
            sb[name] = t
        for s in SIDES:
            for name in ("weR", "beC", "w2T", "b2C", "w3T", "b3F",
                         "wrT", "brR", "wgT", "bgR"):
                key = f"{name}_{s}"
                t = constp.tile(list(inp[key].shape), inp[key].dtype,
                                tag=f"sb_{key}", name=f"sb_{key}")
                nc.sync.dma_start(t[:], inp[key][:])
                sb[key] = t

        one = sb["onesR"][0:1, 0:1]          # [1,1] == 1.0 (f32r)
        ones128F = sb["onesF"][0:1, 0:128]   # [1,128] ones (f32)

        # zUb[side]: precomputed U_loc @ WeU.T + be, [e, m] layout.
        # col block (expert*P + ptile)*MLOC holds tile [128(d), 512(m)].
        zUb = {s: constp.tile([128, K * P * MLOC], F32, tag=f"zUb_{s}",
                              name=f"zUb_{s}")
               for s in SIDES}

        # initial state (step 0): h = c = 0
        zero_col = constp.tile([128, 1], F32R, tag="zero_col", name="zero_col")
        nc.sync.dma_start(zero_col[:], inp["zeroC"][:])
        zero_row = constp.tile([1, 128], F32, tag="zero_row", name="zero_row")
        nc.vector.memset(zero_row[:], 0.0)

        # ---- precompute: the step-invariant expert GEMM ----------------
        with (
            tc.tile_pool(name="prew", bufs=1) as prew,
            tc.tile_pool(name="prepsum", bufs=4, space="PSUM") as prepsum,
        ):
            uloc = prew.tile([128, 2 * MLOC], F32R, tag="ulocT")
            nc.sync.dma_start(uloc[:], inp["ulocT"][:])
            for s in SIDES:
                wet = prew.tile([128, K * 2 * P * 128], F32R, tag="weT")
                nc.sync.dma_start(wet[:], inp[f"weT_{s}"][:])
                for e in range(K):
                    for pt in range(P):
                        ps = prepsum.tile([128, MLOC], F32, tag="zps")
                        for kf in range(2):
                            lcol = ((e * 2 + kf) * P + pt) * 128
                            nc.tensor.matmul(
                                ps[:],
                                wet[:, lcol:lcol + 128],
                                uloc[:, kf * MLOC:(kf + 1) * MLOC],
                                start=(kf == 0), stop=(kf == 1),
                            )
                        blk = (e * P + pt) * MLOC
                        # fold static bias be while copying PSUM -> SBUF
                        nc.vector.tensor_scalar(
                            zUb[s][:, blk:blk + MLOC], ps[:],
                            sb[f"beC_{s}"][:, e * P + pt:e * P + pt + 1],
                            None, ALU.add)

        # ---- per-step pipeline -----------------------------------------
        with (
            tc.tile_pool(name="ctx", bufs=2) as ctxp,
            tc.tile_pool(name="hc", bufs=2) as hcp,
            tc.tile_pool(name="rows", bufs=3) as rowp,
            tc.tile_pool(name="sc", bufs=12) as scp,      # [128,512] scratch
            tc.tile_pool(name="mx", bufs=2) as mxp,
            tc.tile_pool(name="lg", bufs=2) as lgp,
            tc.tile_pool(name="am", bufs=2) as amp,
            tc.tile_pool(name="psg", bufs=1, space="PSUM") as psg,
            tc.tile_pool(name="psw", bufs=3, space="PSUM") as psw,
            tc.tile_pool(name="psl", bufs=1, space="PSUM") as psl,
            tc.tile_pool(name="pss", bufs=3, space="PSUM") as pss,
            tc.tile_pool(name="dramp", bufs=2, space="DRAM") as dramp,
        ):
            h_col = zero_col[:]
            c_row = zero_row[:]
            # static gather offsets for step 0: si=0, ei=M-1
            us_cols = [uTd[:, 0:1], uTd[:, M:M + 1]]
            ue_cols = [uTd[:, M - 1:M], uTd[:, 2 * M - 1:2 * M]]

            for t in range(STEPS):
                # ---- gather us/ue from DRAM into a ctx tile ------------
                ctx = ctxp.tile([128, 4], F32R, tag="ctx")
                for j, ap in enumerate(us_cols + ue_cols):
                    nc.sync.dma_start(ctx[:, j:j + 1], ap)

                # ---- LSTM cell (row layout) ----------------------------
                g_ps = psg.tile([1, 512], F32, tag="g")
                for kx in range(4):
                    nc.tensor.matmul(
                        g_ps[:], ctx[:, kx:kx + 1],
                        sb["wihT"][:, kx * 512:(kx + 1) * 512],
                        start=(kx == 0), stop=False)
                nc.tensor.matmul(g_ps[:], h_col, sb["whhT"][:],
                                 start=False, stop=False)
                nc.tensor.matmul(g_ps[:], one, sb["bihhR"][:],
                                 start=False, stop=True)

                sig_i = rowp.tile([1, 128], F32, tag="sig_i")
                sig_f = rowp.tile([1, 128], F32, tag="sig_f")
                tanh_g = rowp.tile([1, 128], F32, tag="tanh_g")
                sig_o = rowp.tile([1, 128], F32, tag="sig_o")
                nc.scalar.activation(sig_i[:], g_ps[0:1, 0:128], ACTF.Sigmoid)
                nc.scalar.activation(sig_f[:], g_ps[0:1, 128:256], ACTF.Sigmoid)
                nc.scalar.activation(tanh_g[:], g_ps[0:1, 256:384], ACTF.Tanh)
                nc.scalar.activation(sig_o[:], g_ps[0:1, 384:512], ACTF.Sigmoid)

                fc = rowp.tile([1, 128], F32, tag="fc")
                ig = rowp.tile([1, 128], F32, tag="ig")
                c_new = hcp.tile([1, 128], F32, tag="c")
                nc.vector.tensor_tensor(fc[:], sig_f[:], c_row, ALU.mult)
                nc.vector.tensor_tensor(ig[:], sig_i[:], tanh_g[:], ALU.mult)
                nc.vector.tensor_tensor(c_new[:], fc[:], ig[:], ALU.add)
                tanh_c = rowp.tile([1, 128], F32, tag="tanh_c")
                nc.scalar.activation(tanh_c[:], c_new[:], ACTF.Tanh)
                h_row = hcp.tile([1, 128], F32R, tag="h_row")
                nc.vector.tensor_tensor(h_row[:], sig_o[:], tanh_c[:], ALU.mult)
                c_row = c_new[:]

                # h transpose: [1,128] -> [128,1] via PE
                ht_ps = pss.tile([128, 2], F32, tag="pss")
                nc.tensor.matmul(ht_ps[:], h_row[:], sb["onesR"][0:1, 0:2],
                                 start=True, stop=True)
                h_new = hcp.tile([128, 1], F32R, tag="h_col")
                nc.vector.tensor_copy(h_new[:], ht_ps[:, 0:1])
                h_col = h_new[:]

                agin = amp.tile([1, 8], F32, tag="agin")
                if t < STEPS - 1:
                    nc.vector.memset(agin[:], 0.0)

                for si_, s in enumerate(SIDES):
                    ctx_tiles = [h_col, ctx[:, 0:1], ctx[:, 1:2],
                                 ctx[:, 2:3], ctx[:, 3:4]]
                    # ---- r = tanh(Wr @ ctx + br) (row form) ------------
                    r_ps = pss.tile([1, 128], F32, tag="pss")
                    for kc in range(5):
                        nc.tensor.matmul(
                            r_ps[:], ctx_tiles[kc],
                            sb[f"wrT_{s}"][:, kc * 128:(kc + 1) * 128],
                            start=(kc == 0), stop=False)
                    nc.tensor.matmul(r_ps[:], one, sb[f"brR_{s}"][:],
                                     start=False, stop=True)
                    r_row = rowp.tile([1, 128], F32R, tag="r_row")
                    nc.scalar.activation(r_row[:], r_ps[:], ACTF.Tanh)
                    rt_ps = pss.tile([128, 2], F32, tag="pss")
                    nc.tensor.matmul(rt_ps[:], r_row[:], sb["onesR"][0:1, 0:2],
                                     start=True, stop=True)
                    r_col = rowp.tile([128, 1], F32, tag="r_col")
                    nc.vector.tensor_copy(r_col[:], rt_ps[:, 0:1])

                    # ---- gate = softmax(Wg @ ctx + bg) -----------------
                    gt_ps = pss.tile([1, K], F32, tag="pss")
                    for kc in range(5):
                        nc.tensor.matmul(
                            gt_ps[:], ctx_tiles[kc],
                            sb[f"wgT_{s}"][:, kc * K:(kc + 1) * K],
                            start=(kc == 0), stop=False)
                    nc.tensor.matmul(gt_ps[:], one, sb[f"bgR_{s}"][:],
                                     start=False, stop=True)
                    gex = rowp.tile([1, K], F32, tag="gex")
                    nc.scalar.activation(gex[:], gt_ps[:], ACTF.Exp)
                    gsum = rowp.tile([1, 1], F32, tag="gsum")
                    nc.vector.tensor_reduce(gsum[:], gex[:], axis=X, op=ALU.add)
                    grec = rowp.tile([1, 1], F32, tag="grec")
                    nc.vector.reciprocal(grec[:], gsum[:])
                    gate_row = rowp.tile([1, K], F32, tag="gate_row")
                    nc.vector.tensor_scalar(gate_row[:], gex[:], grec[:, 0:1],
                                            None, ALU.mult)
                    gb_ps = pss.tile([128, K], F32, tag="pss")
                    nc.tensor.matmul(gb_ps[:], ones128F[0:1, 0:128],
                                     gate_row[:], start=True, stop=True)
                    gcol = rowp.tile([128, K], F32, tag="gcol")
                    nc.vector.tensor_copy(gcol[:], gb_ps[:])

                    # ---- experts: v, bias+maxout, mixture --------------
                    mx = {}
                    for e in range(K):
                        v_ps = pss.tile([128, P], F32, tag="pss")
                        for pt in range(P):
                            lcol = (e * P + pt) * 128
                            nc.tensor.matmul(
                                v_ps[:, pt:pt + 1],
                                sb[f"weR_{s}"][:, lcol:lcol + 128],
                                r_col[:], start=True, stop=True)
                        vb = rowp.tile([128, P], F32, tag="vb")
                        nc.vector.tensor_copy(vb[:], v_ps[:])

                        zb = []
                        for pt in range(P):
                            blk = (e * P + pt) * MLOC
                            zt = scp.tile([128, MLOC], F32, tag="sc")
                            if pt % 2 == 0:
                                nc.scalar.activation(
                                    zt[:], zUb[s][:, blk:blk + MLOC],
                                    ACTF.Identity, bias=vb[:, pt:pt + 1])
                            else:
                                nc.vector.tensor_scalar(
                                    zt[:], zUb[s][:, blk:blk + MLOC],
                                    vb[:, pt:pt + 1], None, ALU.add)
                            zb.append(zt)
                        pr = [scp.tile([128, MLOC], F32, tag="sc",
                                       name=f"pr{t}{si_}{e}{i}")
                              for i in range(4)]
                        nc.vector.tensor_tensor(pr[0][:], zb[0][:], zb[1][:],
                                                ALU.max)
                        nc.vector.tensor_tensor(pr[1][:], zb[2][:], zb[3][:],
                                                ALU.max)
                        nc.vector.tensor_tensor(pr[2][:], zb[4][:], zb[5][:],
                                                ALU.max)
                        nc.vector.tensor_tensor(pr[3][:], zb[6][:], zb[7][:],
                                                ALU.max)
                        nc.vector.tensor_tensor(pr[0][:], pr[0][:], pr[1][:],
                                                ALU.max)
                        nc.vector.tensor_tensor(pr[2][:], pr[2][:], pr[3][:],
                                                ALU.max)
                        mxe = mxp.tile([128, MLOC], F32, tag=f"mx{e}")
                        nc.vector.tensor_tensor(mxe[:], pr[0][:], pr[2][:],
                                                ALU.max)
                        mx[e] = mxe

                    ga = scp.tile([128, MLOC], F32, tag="sc")
                    gb2 = scp.tile([128, MLOC], F32, tag="sc")
                    nc.vector.tensor_scalar(ga[:], mx[0][:], gcol[:, 0:1],
                                            None, ALU.mult)
                    nc.vector.tensor_scalar(gb2[:], mx[1][:], gcol[:, 1:2],
                                            None, ALU.mult)
                    m1 = mxp.tile([128, MLOC], F32R, tag="m1")
                    nc.vector.tensor_tensor(m1[:], ga[:], gb2[:], ALU.add)

                    # ---- HMN: m2 = maxout(m1 @ W2.T + b2) --------------
                    w2b = []
                    for pt in range(P):
                        ps = psw.tile([128, MLOC], F32, tag="w2ps")
                        nc.tensor.matmul(
                            ps[:],
                            sb[f"w2T_{s}"][:, pt * 128:(pt + 1) * 128],
                            m1[:], start=True, stop=True)
                        zt = scp.tile([128, MLOC], F32, tag="sc")
                        if pt % 2 == 0:
                            nc.scalar.activation(
                                zt[:], ps[:], ACTF.Identity,
                                bias=sb[f"b2C_{s}"][:, pt:pt + 1])
                        else:
                            nc.vector.tensor_scalar(
                                zt[:], ps[:], sb[f"b2C_{s}"][:, pt:pt + 1],
                                None, ALU.add)
                        w2b.append(zt)
                    qr = [scp.tile([128, MLOC], F32, tag="sc",
                                   name=f"qr{t}{si_}{i}")
                          for i in range(4)]
                    nc.vector.tensor_tensor(qr[0][:], w2b[0][:], w2b[1][:],
                                            ALU.max)
                    nc.vector.tensor_tensor(qr[1][:], w2b[2][:], w2b[3][:],
                                            ALU.max)
                    nc.vector.tensor_tensor(qr[2][:], w2b[4][:], w2b[5][:],
                                            ALU.max)
                    nc.vector.tensor_tensor(qr[3][:], w2b[6][:], w2b[7][:],
                                            ALU.max)
                    nc.vector.tensor_tensor(qr[0][:], qr[0][:], qr[1][:],
                                            ALU.max)
                    nc.vector.tensor_tensor(qr[2][:], qr[2][:], qr[3][:],
                                            ALU.max)
                    m2 = mxp.tile([128, MLOC], F32R, tag="m2")
                    nc.vector.tensor_tensor(m2[:], qr[0][:], qr[2][:], ALU.max)

                    # ---- logits: pool dim on the free axis -------------
                    # l_ps [128(m in tile), mt*8+pp]; lhsT = m1/m2 m-slices
                    l_ps = psl.tile([128, 4 * P], F32, tag="lps")
                    for mt in range(4):
                        nc.tensor.matmul(
                            l_ps[:, mt * P:(mt + 1) * P],
                            m1[:, mt * 128:(mt + 1) * 128],
                            sb[f"w3T_{s}"][:, 0:P], start=True, stop=False)
                        nc.tensor.matmul(
                            l_ps[:, mt * P:(mt + 1) * P],
                            m2[:, mt * 128:(mt + 1) * 128],
                            sb[f"w3T_{s}"][:, P:2 * P], start=False, stop=True)
                    lgb = lgp.tile([128, 4 * P], F32, tag="lgb")
                    nc.vector.tensor_tensor(lgb[:], l_ps[:], sb[f"b3F_{s}"][:],
                                            ALU.add)
                    # max over pool pp (innermost, stride 1): [128, 4]
                    lgc = lgp.tile([128, 4], F32, tag="lgc")
                    nc.vector.tensor_reduce(
                        lgc[:], lgb[:].rearrange("p (mt pp) -> p mt pp", pp=P),
                        axis=X, op=ALU.max)
                    # store this side's local logits (host unscrambles
                    # the (p, mt) column order back to m-order)
                    nc.sync.dma_start(out_d[s][t:t + 1, :], lgc[:])

                    # ---- local (max, argmax) over the [128, 4] layout --
                    if t < STEPS - 1:
                        colmax = amp.tile([128, 1], F32, tag="colmax")
                        nc.vector.tensor_reduce(colmax[:], lgc[:], axis=X,
                                                op=ALU.max)
                        rmax_ps = pss.tile([1, 128], F32, tag="pss")
                        nc.tensor.matmul(rmax_ps[:], colmax[:],
                                         sb["ident"][:],
                                         start=True, stop=True)
                        nc.vector.tensor_reduce(
                            agin[0:1, 2 * si_:2 * si_ + 1], rmax_ps[:],
                            axis=X, op=ALU.max)
                        mb_ps = pss.tile([128, 1], F32, tag="pss")
                        nc.tensor.matmul(mb_ps[:], ones128F,
                                         agin[0:1, 2 * si_:2 * si_ + 1],
                                         start=True, stop=True)
                        lmaxb = amp.tile([128, 1], F32, tag="lmaxb")
                        nc.vector.tensor_copy(lmaxb[:], mb_ps[:])
                        msk = amp.tile([128, 4], F32, tag="msk")
                        nc.vector.tensor_scalar(msk[:], lgc[:], lmaxb[:, 0:1],
                                                None, ALU.is_ge)
                        nc.vector.tensor_scalar(msk[:], msk[:], -BIG, BIG,
                                                ALU.mult, ALU.add)
                        nc.vector.tensor_tensor(msk[:], msk[:],
                                                sb["iota2d"][:], ALU.add)
                        colmin = amp.tile([128, 1], F32, tag="colmin")
                        nc.vector.tensor_reduce(colmin[:], msk[:], axis=X,
                                                op=ALU.min)
                        rmin_ps = pss.tile([1, 128], F32, tag="pss")
                        nc.tensor.matmul(rmin_ps[:], colmin[:],
                                         sb["ident"][:],
                                         start=True, stop=True)
                        nc.vector.tensor_reduce(
                            agin[0:1, 2 * si_ + 1:2 * si_ + 2], rmin_ps[:],
                            axis=X, op=ALU.min)

                # ---- AllGather of (max, idx) pairs; global argmax ------
                if t < STEPS - 1 and mode != "static":
                    ag_in = dramp.tile([1, 8], F32, tag="ag_in")
                    ag_out = dramp.tile([8, 8], F32, tag="ag_out")
                    nc.sync.dma_start(ag_in[:], agin[:])
                    nc.gpsimd.collective_compute(
                        "AllGather", ALU.bypass, replica_groups=rg,
                        ins=[ag_in.opt()], outs=[ag_out.opt()])
                    # agb cols: j*8 + rank, j in (max_s, idx_s, max_e, idx_e)
                    agb = amp.tile([1, 32], F32, tag="agb")
                    for j in range(4):
                        nc.sync.dma_start(agb[0:1, j * 8:(j + 1) * 8],
                                          ag_out[:, j:j + 1].transpose([1, 0]))
                    idx4f = amp.tile([1, 4], F32, tag="idx4f")
                    for si_ in range(2):
                        gmax = amp.tile([1, 1], F32, tag="gmax")
                        nc.vector.tensor_reduce(
                            gmax[:], agb[0:1, 16 * si_:16 * si_ + 8],
                            axis=X, op=ALU.max)
                        gmsk = amp.tile([1, 8], F32, tag="gmsk")
                        nc.vector.tensor_scalar(
                            gmsk[:], agb[0:1, 16 * si_:16 * si_ + 8],
                            gmax[:, 0:1], None, ALU.is_ge)
                        nc.vector.tensor_scalar(gmsk[:], gmsk[:], -BIG, BIG,
                                                ALU.mult, ALU.add)
                        nc.vector.tensor_tensor(
                            gmsk[:], gmsk[:],
                            agb[0:1, 16 * si_ + 8:16 * si_ + 16], ALU.add)
                        nc.vector.tensor_reduce(
                            idx4f[0:1, 2 * si_:2 * si_ + 1], gmsk[:],
                            axis=X, op=ALU.min)
                        nc.vector.tensor_scalar(
                            idx4f[0:1, 2 * si_ + 1:2 * si_ + 2],
                            idx4f[0:1, 2 * si_:2 * si_ + 1],
                            float(M), None, ALU.add)
                    idx4i = amp.tile([1, 4], I32, tag="idx4i")
                    nc.vector.tensor_copy(idx4i[:], idx4f[:])
                    if mode != "ag_nold":
                        eng = (ET.SP,)
                        nob = (mode != "ag_assert")
                        si_v = nc.values_load(idx4i[0:1, 0:1], engines=eng,
                                              min_val=0, max_val=M - 1,
                                              skip_runtime_bounds_check=nob)
                        si4_v = nc.values_load(idx4i[0:1, 1:2], engines=eng,
                                               min_val=M, max_val=2 * M - 1,
                                               skip_runtime_bounds_check=nob)
                        ei_v = nc.values_load(idx4i[0:1, 2:3], engines=eng,
                                              min_val=0, max_val=M - 1,
                                              skip_runtime_bounds_check=nob)
                        ei4_v = nc.values_load(idx4i[0:1, 3:4], engines=eng,
                                               min_val=M, max_val=2 * M - 1,
                                               skip_runtime_bounds_check=nob)
                    if mode == "full":
                        us_cols = [uTd[:, bass.ds(si_v, 1)],
                                   uTd[:, bass.ds(si4_v, 1)]]
                        ue_cols = [uTd[:, bass.ds(ei_v, 1)],
                                   uTd[:, bass.ds(ei4_v, 1)]]

    nc.compile()
    return nc


def _pack_inputs(full):
    """Split/transform full inputs into 8 per-core input maps."""
    U = np.ascontiguousarray(np.asarray(full["U"], np.float32)[0])  # (M, 2D)
    d = D
    common = {}
    # uTd: [p, j*M + m] = U[m, j*128 + p]
    uTd = np.empty((128, 2 * M), np.float32)
    for j in range(2):
        uTd[:, j * M:(j + 1) * M] = U[:, j * 128:(j + 1) * 128].T
    common["uTd"] = uTd
    Wih = np.asarray(full["lstm_Wih"], np.float32)    # (512, 512)
    WihT = Wih.T                                      # [x, g]
    wihT = np.empty((128, 4 * 512), np.float32)
    for kx in range(4):
        wihT[:, kx * 512:(kx + 1) * 512] = WihT[kx * 128:(kx + 1) * 128, :]
    common["wihT"] = wihT
    common["whhT"] = np.ascontiguousarray(
        np.asarray(full["lstm_Whh"], np.float32).T)   # (128, 512)
    common["bihhR"] = (np.asarray(full["lstm_bih"], np.float32)
                       + np.asarray(full["lstm_bhh"], np.float32))[None, :]
    common["onesR"] = np.ones((1, 512), np.float32)
    common["onesF"] = np.ones((1, 512), np.float32)
    common["zeroC"] = np.zeros((128, 1), np.float32)
    common["ident"] = np.eye(128, dtype=np.float32)

    for s in SIDES:
        We = np.asarray(full[f"We_{s}"], np.float32)      # (K, P*D, 3D)
        be = np.asarray(full[f"be_{s}"], np.float32)      # (K, P*D)
        weT = np.empty((128, K * 2 * P * 128), np.float32)
        weR = np.empty((128, K * P * 128), np.float32)
        beC = np.empty((128, K * P), np.float32)
        for e in range(K):
            for kf in range(2):
                for pt in range(P):
                    col = ((e * 2 + kf) * P + pt) * 128
                    # lhsT[f, ec] = We[e, pt*128+ec, kf*128+f]
                    weT[:, col:col + 128] = We[e, pt * 128:(pt + 1) * 128,
                                               kf * 128:(kf + 1) * 128].T
            for pt in range(P):
                col = (e * P + pt) * 128
                weR[:, col:col + 128] = We[e, pt * 128:(pt + 1) * 128,
                                           2 * d:3 * d].T
                beC[:, e * P + pt] = be[e, pt * 128:(pt + 1) * 128]
        common[f"weT_{s}"] = weT
        common[f"weR_{s}"] = weR
        common[f"beC_{s}"] = beC

        W2 = np.asarray(full[f"W2_{s}"], np.float32)      # (P*D, D)
        w2T = np.empty((128, P * 128), np.float32)
        b2C = np.empty((128, P), np.float32)
        b2 = np.asarray(full[f"b2_{s}"], np.float32)
        for pt in range(P):
            w2T[:, pt * 128:(pt + 1) * 128] = W2[pt * 128:(pt + 1) * 128, :].T
            b2C[:, pt] = b2[pt * 128:(pt + 1) * 128]
        common[f"w2T_{s}"] = w2T
        common[f"b2C_{s}"] = b2C

        W3 = np.asarray(full[f"W3_{s}"], np.float32)      # (P, 2D)
        w3T = np.empty((128, 2 * P), np.float32)
        for kf in range(2):
            w3T[:, kf * P:(kf + 1) * P] = W3[:, kf * 128:(kf + 1) * 128].T
        common[f"w3T_{s}"] = w3T
        b3 = np.asarray(full[f"b3_{s}"], np.float32)
        common[f"b3F_{s}"] = np.broadcast_to(
            np.tile(b3, 4)[None, :], (128, 4 * P)).copy()

        Wr = np.asarray(full[f"Wr_{s}"], np.float32)      # (D, 5D)
        WrT = Wr.T                                        # [ctx, i]
        wrT = np.empty((128, 5 * 128), np.float32)
        for kc in range(5):
            wrT[:, kc * 128:(kc + 1) * 128] = WrT[kc * 128:(kc + 1) * 128, :]
        common[f"wrT_{s}"] = wrT
        common[f"brR_{s}"] = np.asarray(full[f"br_{s}"], np.float32)[None, :]

        Wg = np.asarray(full[f"Wg_{s}"], np.float32)      # (K, 5D)
        WgT = Wg.T
        wgT = np.empty((128, 5 * K), np.float32)
        for kc in range(5):
            wgT[:, kc * K:(kc + 1) * K] = WgT[kc * 128:(kc + 1) * 128, :]
        common[f"wgT_{s}"] = wgT
        common[f"bgR_{s}"] = np.asarray(full[f"bg_{s}"], np.float32)[None, :]

    in_maps = []
    for c in range(NCORES):
        m = dict(common)
        io = np.empty((128, 4), np.float32)
        for mt in range(4):
            io[:, mt] = c * MLOC + mt * 128 + np.arange(128)
        m["iota2d"] = io
        ulocT = np.empty((128, 2 * MLOC), np.float32)
        for j in range(2):
            ulocT[:, j * MLOC:(j + 1) * MLOC] = \
                U[c * MLOC:(c + 1) * MLOC, j * 128:(j + 1) * 128].T
        m["ulocT"] = ulocT
        in_maps.append(m)
    return in_maps


def kernel(**inputs):
    if "nc" not in _CACHE:
        _CACHE["nc"] = _build()
    nc = _CACHE["nc"]
    in_maps = _pack_inputs(inputs)
    res = bass_utils.run_bass_kernel_spmd(
        nc, in_maps, core_ids=list(range(NCORES)))
    starts = np.empty((1, STEPS, M), np.float32)
    ends = np.empty((1, STEPS, M), np.float32)
    for c in range(NCORES):
        for dst, key in ((starts, "out_s"), (ends, "out_e")):
            raw = res.results[c][key]                       # [4, 512] (p,mt)
            dst[0, :, c * MLOC:(c + 1) * MLOC] = (
                raw.reshape(STEPS, 128, 4).transpose(0, 2, 1)
                .reshape(STEPS, MLOC))
    return starts, ends
